# revision 8
# baseline (speedup 1.0000x reference)
"""Trainium2 Bass kernel for HierarchicalCSNet (8 groups, 256x256, G-fused chain).

v2: spatial row-sharding across 8 NeuronCores (as v1) with three PE-side
restructurings validated by microbenchmark:
  - all feature buffers / weights in bf16 (PE streams bf16 ~6% faster than
    f32r and every SBUF copy halves);
  - tails (t2..t4) computed as 1-row pairs in a column-tiled [128,256] PSUM:
    3 MMs M=64 -> P[0:64] (dy0/dy1 of row r), 3 MMs M=64 -> P[64:128]
    (dy0/dy1 of row r+1), 3 MMs M=128 block-diag (dy2 of both rows straight
    off the stacked plane) = 9 N=256 matmuls per 2 rows instead of 12
    N=256-equivalents, and ONE [128,256] PReLU writes the plain AND stacked
    planes of the destination slot in a single activation;
  - t5 (64->1 conv) as M=3-per-dx matmuls + tiny ones-combine matmul:
    2 N=256 MMs per row + 1 combine per 4 rows (9 MMs/4 rows vs 24).

Slot grid per core: slot s in [0,56) <-> global row 32c - 12 + s. Feature
rows at pitch 258 (zero pad col each side). Margins as v1.
"""
import sys, os
import numpy as np
import ml_dtypes

for _p in ("/opt/trn_rl_repo", os.path.expanduser("~/.axon_site/_ro/trn_rl_repo")):
    if os.path.isdir(_p) and _p not in sys.path:
        sys.path.append(_p)

G, BS = 8, 32
H = W = 256
PITCH = 258
NSLOT = 54          # slots [1,55) stored in F/TA (row = slot-1)
TB_BASE = 9
TB_ROWS = 38        # slots [9,47) stored in TB (row = slot-9)


def _h_range(m):
    return (1, 55) if m == 0 else (m, 56 - m)


def _fus_range(m):
    return (m + 1, 55 - m)


def _chunks():
    """(m, s0) list for h-conv tiles, in program order."""
    out = []
    for m in range(G):
        lo, hi = _h_range(m)
        for s0 in range(lo, hi, 2):
            out.append((m, s0))
    return out


_CHUNKS = _chunks()
NCHUNK = len(_CHUNKS)

_BUILT = None


def _build_program(reps=1):
    import concourse.bacc as bacc
    import concourse.mybir as mybir
    import concourse.tile as tile

    f32 = mybir.dt.float32
    f32r = mybir.dt.float32r
    bf16 = mybir.dt.bfloat16
    PRELU = mybir.ActivationFunctionType.Prelu
    COPY = mybir.ActivationFunctionType.Copy

    nc = bacc.Bacc("TRN2", target_bir_lowering=False)
    r9_d = nc.dram_tensor("r9", [9, NCHUNK * 516], bf16, kind="ExternalInput")
    wh_d = nc.dram_tensor("wh", [9, G * 64], bf16, kind="ExternalInput")
    wf_d = nc.dram_tensor("wf", [7 * 128, 576], bf16, kind="ExternalInput")
    wt_d = nc.dram_tensor("wt", [G * 128, 1728], bf16, kind="ExternalInput")
    w5a_d = nc.dram_tensor("w5a", [128, G * 3], bf16, kind="ExternalInput")
    w5b_d = nc.dram_tensor("w5b", [64, G * 3], bf16, kind="ExternalInput")
    cb_d = nc.dram_tensor("cb", [128, 6], bf16, kind="ExternalInput")
    bb_d = nc.dram_tensor("bb", [64, 15], f32, kind="ExternalInput")
    aa_d = nc.dram_tensor("aa", [64, 15], f32, kind="ExternalInput")
    bt_d = nc.dram_tensor("bt", [128, G * 3], f32, kind="ExternalInput")
    at_d = nc.dram_tensor("at", [128, G * 3], f32, kind="ExternalInput")
    mm_d = nc.dram_tensor("mm", [128, 2], f32, kind="ExternalInput")
    o_d = nc.dram_tensor("o", [G * 32, 256], f32, kind="ExternalOutput")

    with tile.TileContext(nc) as tc:
        with tc.tile_pool(name="const", bufs=1) as cst, \
             tc.tile_pool(name="big", bufs=1) as big, \
             tc.tile_pool(name="wfp", bufs=2) as wfp, \
             tc.tile_pool(name="wtp", bufs=2) as wtp, \
             tc.tile_pool(name="r9p", bufs=1) as r9p, \
             tc.tile_pool(name="o5p", bufs=2) as o5p, \
             tc.tile_pool(name="sqp", bufs=3) as sqp, \
             tc.tile_pool(name="psA", bufs=3, space="PSUM") as psA, \
             tc.tile_pool(name="psT", bufs=3, space="PSUM") as psT:

            wh_t = cst.tile([9, G * 64], bf16)
            w5a_t = cst.tile([128, G * 3], bf16)
            w5b_t = cst.tile([64, G * 3], bf16)
            cb_t = cst.tile([128, 6], bf16)
            bb_t = cst.tile([64, 15], f32)
            aa_t = cst.tile([64, 15], f32)
            bt_t = cst.tile([128, G * 3], f32)
            at_t = cst.tile([128, G * 3], f32)
            mm_t = cst.tile([128, 2], f32)
            F = big.tile([128, NSLOT * PITCH], bf16)
            TA = big.tile([128, NSLOT * PITCH], bf16)
            TB = big.tile([128, TB_ROWS * PITCH], bf16)

            nc.sync.dma_start(wh_t[:], wh_d[:])
            nc.sync.dma_start(w5a_t[:], w5a_d[:])
            nc.sync.dma_start(w5b_t[:], w5b_d[:])
            nc.sync.dma_start(cb_t[:], cb_d[:])
            nc.sync.dma_start(bb_t[:], bb_d[:])
            nc.sync.dma_start(aa_t[:], aa_d[:])
            nc.sync.dma_start(bt_t[:], bt_d[:])
            nc.sync.dma_start(at_t[:], at_d[:])
            nc.sync.dma_start(mm_t[:], mm_d[:])
            nc.vector.memset(TA[:, 0:27 * PITCH].bitcast(f32), 0.0)
            nc.gpsimd.memset(TA[:, 27 * PITCH:].bitcast(f32), 0.0)
            nc.gpsimd.memset(F[:].bitcast(f32), 0.0)
            nc.gpsimd.memset(TB[:].bitcast(f32), 0.0)
            for _si in range(3):
                _sq = sqp.tile([128, 258], bf16, tag="sq", name="sq_init")
                nc.vector.memset(_sq[:].bitcast(f32), 0.0)

            Fv = F[:].rearrange("p (r x) -> p r x", x=PITCH)
            TAv = TA[:].rearrange("p (r x) -> p r x", x=PITCH)
            TBv = TB[:].rearrange("p (r x) -> p r x", x=PITCH)

            def mask(view, base, mlo, mhi, stacked=False, nrows=NSLOT):
                # zero out-of-image rows: top slots [mlo,12) with mm[:,0],
                # bottom slots [44,mhi) with mm[:,1] (no-op on interior cores)
                for (lo, hi, col) in ((mlo, 12, 0), (44, mhi, 1)):
                    if hi <= lo:
                        continue
                    nc.vector.tensor_scalar_mul(
                        view[0:64, lo - base:hi - base, :],
                        view[0:64, lo - base:hi - base, :],
                        mm_t[0:64, col:col + 1])
                if not stacked:
                    return
                # upper half holds rows shifted by +1 slot
                for (lo, hi, col) in ((mlo, 12, 0), (44, mhi, 1)):
                    rlo = max(0, lo - base - 1)
                    rhi = min(nrows, hi - base - 1)
                    if rhi <= rlo:
                        continue
                    nc.vector.tensor_scalar_mul(
                        view[64:128, rlo:rhi, :],
                        view[64:128, rlo:rhi, :],
                        mm_t[64:128, col:col + 1])

            def stack_dma(buf, base, s0, nrows):
                # buf[64:128, r] := buf[0:64, r+1] for the rows enabled by the
                # freshly written tile (slots s0, s0+1)
                d0 = max(0, s0 - base - 1)
                d1 = min(nrows - 1, s0 - base + 1)
                if d1 <= d0:
                    return
                nc.sync.dma_start(
                    buf[64:128, d0 * PITCH:d1 * PITCH],
                    buf[0:64, (d0 + 1) * PITCH:(d1 + 1) * PITCH])

            def make_h_thunks(m):
                """One closure per h tile of group m (as v1)."""
                h_lo, h_hi = _h_range(m)
                hdst, hbase = (TAv, 1) if m == 0 else (Fv, 1)
                s0s = list(range(h_lo, h_hi, 2))
                thunks = []
                state = {}

                def prefetch():
                    if "v" in state:
                        return
                    r9s = r9p.tile([9, 27 * 516], bf16, tag="r9", name="r9s")
                    ck = chunk_base[m]
                    nc.sync.dma_start(
                        r9s[0:9, 0:len(s0s) * 516],
                        r9_d[0:9, ck * 516:(ck + len(s0s)) * 516])
                    state["v"] = r9s[:].rearrange(
                        "p (b r x) -> p b r x", r=2, x=258)

                for i, s0 in enumerate(s0s):
                    def thunk(i=i, s0=s0, m=m, hdst=hdst, hbase=hbase):
                        prefetch()
                        r9v = state["v"]
                        pt = psA.tile([64, 512], f32, tag="ps")
                        nc.tensor.matmul(pt[:], wh_t[:, m * 64:(m + 1) * 64],
                                         r9v[0:9, i:i + 1, 0:2, 1:257],
                                         start=True, stop=True)
                        nc.scalar.activation(
                            hdst[0:64, s0 - hbase:s0 - hbase + 2, 1:257], pt[:],
                            PRELU, bias=bb_t[:, m:m + 1], scale=1.0,
                            alpha=aa_t[:, m:m + 1])
                        if m == 0:
                            stack_dma(TA, 1, s0, NSLOT)
                    thunks.append(thunk)

                def finish(m=m, hdst=hdst, hbase=hbase, h_lo=h_lo, h_hi=h_hi):
                    mask(hdst, hbase, h_lo, h_hi, stacked=(m == 0))
                return thunks, finish, prefetch

            chunk_base = []
            k0 = 0
            for m in range(G):
                chunk_base.append(k0)
                lo, hi = _h_range(m)
                k0 += len(range(lo, hi, 2))

            for _rep in range(reps):
              pending_h = None      # (thunks, finish, prefetch) for group m+1
              wt_next = [None]
              wf_next = [None]
              for m in range(G):
                # --- this group's tail weights (prefetched in m-1's tails) ---
                if wt_next[0] is not None:
                    wt_t = wt_next[0]
                else:
                    wt_t = wtp.tile([128, 1728], bf16, tag="wt")
                    nc.sync.dma_start(wt_t[:], wt_d[m * 128:(m + 1) * 128, :])

                # --- h_m (m=0 runs standalone; m>=1 prefetched in m-1) ---
                if m == 0:
                    thunks, finish, pre = make_h_thunks(0)
                    pre()
                    for t in thunks:
                        t()
                    finish()
                else:
                    pending_thunks, pending_finish, _ = pending_h
                    for t in pending_thunks:
                        t()
                    pending_finish()
                if m < G - 1:
                    pending_h = make_h_thunks(m + 1)
                else:
                    pending_h = ([], None, None)

                # --- fusion m (m>=1): K=128 from F = [h_m | feature_{m-1}] ---
                if m >= 1:
                    if wf_next[0] is not None:
                        wf_t = wf_next[0]
                    else:
                        wf_t = wfp.tile([128, 576], bf16, tag="wf")
                        nc.sync.dma_start(
                            wf_t[:], wf_d[(m - 1) * 128:m * 128, :])
                    f_lo, f_hi = _fus_range(m)
                    for s0 in range(f_lo, f_hi, 2):
                        pt = psA.tile([64, 512], f32, tag="ps")
                        for t in range(9):
                            dy, dx = t // 3, t % 3
                            rr = s0 + dy - 1 - 1
                            nc.tensor.matmul(
                                pt[:], wf_t[:, t * 64:(t + 1) * 64],
                                Fv[0:128, rr:rr + 2, dx:dx + 256],
                                start=(t == 0), stop=(t == 8))
                        nc.scalar.activation(
                            TAv[0:64, s0 - 1:s0 + 1, 1:257], pt[:],
                            PRELU, bias=bb_t[:, 8 + m - 1:8 + m],
                            scale=1.0, alpha=aa_t[:, 8 + m - 1:8 + m])
                        stack_dma(TA, 1, s0, NSLOT)
                    mask(TAv, 1, f_lo, f_hi, stacked=True)

                # --- feature_m (in TA) -> F[64:128] for next fusion ---
                if m < G - 1:
                    lo, hi = (1, 55) if m == 0 else _fus_range(m)
                    nc.sync.dma_start(
                        F[64:128, (lo - 1) * PITCH:(hi - 1) * PITCH],
                        TA[0:64, (lo - 1) * PITCH:(hi - 1) * PITCH])

                # --- prefetch next group's weights + first r9 batches ---
                wt_next[0] = wf_next[0] = None
                if m < G - 1:
                    wt_next[0] = wtp.tile([128, 1728], bf16, tag="wt",
                                          name="wt_n")
                    nc.sync.dma_start(
                        wt_next[0][:], wt_d[(m + 1) * 128:(m + 2) * 128, :])
                    wf_next[0] = wfp.tile([128, 576], bf16, tag="wf",
                                          name="wf_n")
                    nc.sync.dma_start(
                        wf_next[0][:], wf_d[m * 128:(m + 1) * 128, :])
                    pending_h[2]()

                # --- tails: 1-row pairs, col-tiled [128,256] PSUM, 9 MMs ---
                def tconv(src_v, src_base, dst_v, dst_base, dst_buf, dst_rows,
                          lo, hi, cv, bcol):
                    cbase = cv * 576
                    for s0 in range(lo, hi, 2):
                        pt = psT.tile([128, 256], f32, tag="psT", name="ptt")
                        for dx in range(3):   # dy0/dy1 of row s0
                            nc.tensor.matmul(
                                pt[0:64, :],
                                wt_t[:, cbase + dx * 64:cbase + dx * 64 + 64],
                                src_v[0:128, s0 - 1 - src_base:s0 - src_base,
                                      dx:dx + 256],
                                start=(dx == 0), stop=False)
                        for dx in range(3):   # dy0/dy1 of row s0+1
                            nc.tensor.matmul(
                                pt[64:128, :],
                                wt_t[:, cbase + dx * 64:cbase + dx * 64 + 64],
                                src_v[0:128, s0 - src_base:s0 + 1 - src_base,
                                      dx:dx + 256],
                                start=(dx == 0), stop=False)
                        for dx in range(3):   # dy2 of both rows (block-diag)
                            c0 = cbase + 192 + dx * 128
                            nc.tensor.matmul(
                                pt[:],
                                wt_t[:, c0:c0 + 128],
                                src_v[0:128, s0 + 1 - src_base:
                                      s0 + 2 - src_base, dx:dx + 256],
                                start=False, stop=(dx == 2))
                        # one PReLU writes plain(s0) [0:64] + stacked(s0) [64:128]
                        nc.scalar.activation(
                            dst_v[0:128, s0 - dst_base:s0 - dst_base + 1,
                                  1:257], pt[:],
                            PRELU, bias=bt_t[:, bcol:bcol + 1], scale=1.0,
                            alpha=at_t[:, bcol:bcol + 1])
                        # stitch: stacked(s0-1) <- row s0, plain(s0+1) <- row s0+1
                        if s0 - 1 >= dst_base:
                            nc.sync.dma_start(
                                dst_buf[64:128,
                                        (s0 - 1 - dst_base) * PITCH:
                                        (s0 - dst_base) * PITCH],
                                dst_buf[0:64,
                                        (s0 - dst_base) * PITCH:
                                        (s0 + 1 - dst_base) * PITCH])
                        if s0 + 1 < dst_base + dst_rows:
                            nc.gpsimd.dma_start(
                                dst_buf[0:64,
                                        (s0 + 1 - dst_base) * PITCH:
                                        (s0 + 2 - dst_base) * PITCH],
                                dst_buf[64:128,
                                        (s0 - dst_base) * PITCH:
                                        (s0 + 1 - dst_base) * PITCH])

                tconv(TAv, 1, TBv, TB_BASE, TB, TB_ROWS, 9, 47, 0, m * 3 + 0)
                mask(TBv, TB_BASE, 9, 47, stacked=True, nrows=TB_ROWS)
                tconv(TBv, TB_BASE, TAv, 1, TA, NSLOT, 10, 46, 1, m * 3 + 1)
                mask(TAv, 1, 10, 46, stacked=True)
                tconv(TAv, 1, TBv, TB_BASE, TB, TB_ROWS, 11, 45, 2, m * 3 + 2)
                mask(TBv, TB_BASE, 11, 45, stacked=True, nrows=TB_ROWS)

                # --- t5: M=3 per row-half (dy01+dy2) -> Sraw via ACT, then 3
                # ones-combine matmuls realign per-dx partials via rhs col
                # windows. Combines are software-pipelined one pair behind the
                # MMs so the PE never waits on the ACT extraction chain. ---
                def t5_combine(st):
                    Sq_, r_ = st
                    pc = psT.tile([128, 256], f32, tag="pc", name="pc5", bufs=2)
                    for dx in range(3):
                        nc.tensor.matmul(pc[0:2, :],
                                         cb_t[:, dx * 2:dx * 2 + 2],
                                         Sq_[0:128, dx:dx + 256],
                                         start=(dx == 0), stop=(dx == 2))
                    o5 = o5p.tile([2, 256], f32, tag="o5")
                    nc.scalar.activation(o5[:], pc[0:2, :], COPY)
                    nc.gpsimd.dma_start(
                        o_d[m * 32 + (r_ - 12):m * 32 + (r_ - 12) + 2, :],
                        o5[:])

                t5_prev = None
                for r in range(12, 44, 2):
                    pt = psT.tile([128, 256], f32, tag="psT", name="pt5")
                    for half, row in ((0, r), (64, r + 1)):
                        nc.tensor.matmul(
                            pt[half:half + 3, :],
                            w5a_t[:, m * 3:m * 3 + 3],
                            TBv[0:128, row - 1 - TB_BASE:row - TB_BASE, 1:257],
                            start=True, stop=False)
                        nc.tensor.matmul(
                            pt[half:half + 3, :],
                            w5b_t[:, m * 3:m * 3 + 3],
                            TBv[0:64, row + 1 - TB_BASE:row + 2 - TB_BASE,
                                1:257],
                            start=False, stop=True)
                    Sq = sqp.tile([128, 258], bf16, tag="sq", name="sq")
                    nc.scalar.activation(Sq[0:3, 1:257], pt[0:3, :], COPY)
                    nc.scalar.activation(Sq[64:67, 1:257], pt[64:67, :], COPY)
                    if t5_prev is not None:
                        t5_combine(t5_prev)
                    t5_prev = (Sq, r)
                t5_combine(t5_prev)

    nc.compile()
    return nc


def _get_program():
    global _BUILT
    if _BUILT is None:
        _BUILT = _build_program()
    return _BUILT


def _host_heads(x, sample_w, up_w, up_b):
    """r[m] (256x256) for all groups, float32."""
    X = x[0, 0].reshape(8, 32, 8, 32).astype(np.float64)
    R = np.empty((G, H, W), np.float32)
    for m in range(G):
        S = np.einsum('ipjq,cpq->cij', X, sample_w[m, :, 0].astype(np.float64))
        U = np.einsum('cij,uc->uij', S, up_w[m, :, :, 0, 0].astype(np.float64))
        U = U + up_b[m].astype(np.float64)[:, None, None]
        R[m] = U.reshape(32, 32, 8, 8).transpose(2, 0, 3, 1).reshape(256, 256)
    return R


def _build_r9(R):
    """Per-core prestacked h-conv rhs: [8][NCHUNK*9, 516] float32."""
    from numpy.lib.stride_tricks import sliding_window_view
    rp = np.zeros((G, H + 26, W + 4), np.float32)   # rows g+13, cols x+2
    rp[:, 13:13 + H, 2:2 + W] = R
    out = np.empty((8, NCHUNK, 9, 516), np.float32)
    k0 = 0
    for m in range(G):
        lo, hi = _h_range(m)
        s0s = np.arange(lo, hi, 2)
        SW = sliding_window_view(rp[m], (2, 258))
        for t in range(9):
            dy, dx = t // 3, t % 3
            g0 = (32 * np.arange(8))[:, None] + s0s[None, :] + dy
            out[:, k0:k0 + len(s0s), t] = SW[g0, dx].reshape(8, len(s0s), 516)
        k0 += len(s0s)
    # device layout: [9 partitions, NCHUNK*516] so several chunks come per DMA
    return np.ascontiguousarray(out.transpose(0, 2, 1, 3)).reshape(
        8, 9, NCHUNK * 516)


_EXEC = None


def _get_executor():
    """Persistent jitted shard_map executor over 8 cores."""
    global _EXEC
    if _EXEC is not None:
        return _EXEC
    nc = _get_program()
    _EXEC = _make_executor(nc)
    return _EXEC


def _prep_device_args(in_maps):
    import jax
    sharded, in_names, out_names, zero_shapes = _get_executor()
    concat_in = [np.concatenate([in_maps[c][n] for c in range(8)], axis=0)
                 for n in in_names]
    concat_zero = [np.zeros((8 * s[0],) + tuple(s[1:]), d)
                   for (s, d) in zero_shapes]
    return [jax.device_put(a) for a in concat_in + concat_zero]


def _run(in_maps):
    sharded, in_names, out_names, zero_shapes = _get_executor()
    args = _prep_device_args(in_maps)
    outs = sharded(*args)
    res = []
    for c in range(8):
        res.append({n: np.asarray(outs[i]).reshape((8,) + zero_shapes[i][0])[c]
                    for i, n in enumerate(out_names)})
    return res


def bench(in_maps, iters=5):
    """Device-resident repeat timing of the sharded program."""
    import time as _t
    sharded, *_ = _get_executor()
    args = _prep_device_args(in_maps)
    r = sharded(*args)
    [x.block_until_ready() for x in r]
    times = []
    for _ in range(iters):
        t0 = _t.perf_counter()
        r = sharded(*args)
        [x.block_until_ready() for x in r]
        times.append(_t.perf_counter() - t0)
    return min(times), times


def _make_executor(nc):
    import jax
    from jax.sharding import Mesh, PartitionSpec
    from jax.experimental.shard_map import shard_map
    from concourse import bass2jax
    import concourse.mybir as mybir

    bass2jax.install_neuronx_cc_hook()
    part_name = nc.partition_id_tensor.name if nc.partition_id_tensor else None
    in_names, out_names, out_avals, zero_shapes = [], [], [], []
    for alloc in nc.m.functions[0].allocations:
        if not isinstance(alloc, mybir.MemoryLocationSet):
            continue
        name = alloc.memorylocations[0].name
        if alloc.kind == "ExternalInput":
            if name != part_name:
                in_names.append(name)
        elif alloc.kind == "ExternalOutput":
            out_names.append(name)
            shape = tuple(alloc.tensor_shape)
            dtype = mybir.dt.np(alloc.dtype)
            out_avals.append(jax.core.ShapedArray(shape, dtype))
            zero_shapes.append((shape, dtype))
    all_names = in_names + out_names + ([part_name] if part_name else [])

    def _body(*args):
        operands = list(args)
        if part_name:
            operands.append(bass2jax.partition_id_tensor())
        return tuple(bass2jax._bass_exec_p.bind(
            *operands, out_avals=tuple(out_avals), in_names=tuple(all_names),
            out_names=tuple(out_names), lowering_input_output_aliases=(),
            sim_require_finite=True, sim_require_nnan=True, nc=nc))

    mesh = Mesh(np.asarray(jax.devices()[:8]), ("core",))
    n = len(in_names) + len(out_names)
    sharded = jax.jit(shard_map(_body, mesh=mesh,
                                in_specs=(PartitionSpec("core"),) * n,
                                out_specs=(PartitionSpec("core"),) * len(out_names),
                                check_rep=False), keep_unused=True)
    return sharded, in_names, out_names, zero_shapes


def bench_reps(in_maps, iters=5):
    """Time a 2x-unrolled variant against the 1x program; the difference is
    one full device execution, free of fixed dispatch overhead."""
    import time as _t
    import jax
    results = {}
    for reps in (1, 2):
        nc = _get_program() if reps == 1 else _build_program(reps=2)
        sharded, in_names, out_names, zero_shapes = _make_executor(nc)
        concat_in = [np.concatenate([in_maps[c][n] for c in range(8)], axis=0)
                     for n in in_names]
        concat_zero = [np.zeros((8 * s[0],) + tuple(s[1:]), d)
                       for (s, d) in zero_shapes]
        args = [jax.device_put(a) for a in concat_in + concat_zero]
        r = sharded(*args); [x.block_until_ready() for x in r]
        ts = []
        for _ in range(iters):
            t0 = _t.perf_counter()
            r = sharded(*args)
            [x.block_until_ready() for x in r]
            ts.append(_t.perf_counter() - t0)
        ts.sort()
        results[reps] = ts
    import statistics
    d = statistics.median(results[2]) - statistics.median(results[1])
    return max(d, 0.0), results


def build_in_maps(x, sample_w, up_w, up_b, h1_w, h1_b, h1_a, fus_w, fus_b,
                  fus_a, t2_w, t2_b, t2_a, t3_w, t3_b, t3_a, t4_w, t4_b,
                  t4_a, t5_w, t5_b):
    bfl = ml_dtypes.bfloat16

    R = _host_heads(x, sample_w, up_w, up_b)
    r9 = _build_r9(R).astype(bfl)

    wh = np.ascontiguousarray(
        h1_w[:, :, 0].reshape(G, 64, 9).transpose(2, 0, 1).reshape(9, G * 64)
    ).astype(bfl)
    # fusion lhsT rows 0:64 <- h weights (cat idx 64:128), rows 64:128 <- feature
    wf = np.empty((7, 128, 9, 64), np.float32)
    for mm1 in range(7):
        for t in range(9):
            wf[mm1, 0:64, t] = fus_w[mm1, :, 64:128, t // 3, t % 3].T
            wf[mm1, 64:128, t] = fus_w[mm1, :, 0:64, t // 3, t % 3].T
    wf = wf.reshape(7 * 128, 576).astype(bfl)

    # tails: per (group, cv): cols [cv*576 + dx*64] dy01 lhsT (M=64),
    # cols [cv*576 + 192 + dx*128] dy2 block-diag lhsT (M=128)
    wt = np.zeros((G, 128, 3, 576), np.float32)
    for m in range(G):
        for cv, tw in enumerate((t2_w, t3_w, t4_w)):
            for dx in range(3):
                wt[m, 0:64, cv, dx * 64:dx * 64 + 64] = tw[m, :, :, 0, dx].T
                wt[m, 64:128, cv, dx * 64:dx * 64 + 64] = tw[m, :, :, 1, dx].T
                c2 = 192 + dx * 128
                wt[m, 0:64, cv, c2:c2 + 64] = tw[m, :, :, 2, dx].T
                wt[m, 64:128, cv, c2 + 64:c2 + 128] = tw[m, :, :, 2, dx].T
    wt = wt.reshape(G * 128, 1728).astype(bfl)

    w5a = np.zeros((128, G * 3), np.float32)
    w5b = np.zeros((64, G * 3), np.float32)
    for m in range(G):
        for dx in range(3):
            w5a[0:64, m * 3 + dx] = t5_w[m, 0, :, 0, dx]
            w5a[64:128, m * 3 + dx] = t5_w[m, 0, :, 1, dx]
            w5b[:, m * 3 + dx] = t5_w[m, 0, :, 2, dx]
    w5a = w5a.astype(bfl)
    w5b = w5b.astype(bfl)

    cb = np.zeros((128, 6), np.float32)
    for dx in range(3):
        cb[dx, dx * 2] = 1.0
        cb[64 + dx, dx * 2 + 1] = 1.0
    cb = cb.astype(bfl)

    bb = np.zeros((64, 15), np.float32)
    aa = np.zeros((64, 15), np.float32)
    bb[:, 0:8] = h1_b.T; aa[:, 0:8] = np.broadcast_to(h1_a, (64, 8))
    bb[:, 8:15] = fus_b.T; aa[:, 8:15] = np.broadcast_to(fus_a, (64, 7))
    bt = np.zeros((128, G * 3), np.float32)
    at = np.zeros((128, G * 3), np.float32)
    for m in range(G):
        for cv, (tb, ta) in enumerate(((t2_b, t2_a), (t3_b, t3_a),
                                       (t4_b, t4_a))):
            bt[0:64, m * 3 + cv] = tb[m]
            bt[64:128, m * 3 + cv] = tb[m]
            at[:, m * 3 + cv] = ta[m]

    in_maps = []
    for c in range(8):
        mmk = np.ones((128, 2), np.float32)
        if c == 0:
            mmk[:, 0] = 0.0
        if c == 7:
            mmk[:, 1] = 0.0
        in_maps.append({"r9": r9[c], "wh": wh, "wf": wf, "wt": wt,
                        "w5a": w5a, "w5b": w5b, "cb": cb,
                        "bb": bb, "aa": aa, "bt": bt, "at": at,
                        "mm": mmk})
    return in_maps


def kernel(x, sample_w, up_w, up_b, h1_w, h1_b, h1_a, fus_w, fus_b, fus_a,
           t2_w, t2_b, t2_a, t3_w, t3_b, t3_a, t4_w, t4_b, t4_a, t5_w, t5_b):
    in_maps = build_in_maps(
        x, sample_w, up_w, up_b, h1_w, h1_b, h1_a, fus_w, fus_b, fus_a,
        t2_w, t2_b, t2_a, t3_w, t3_b, t3_a, t4_w, t4_b, t4_a, t5_w, t5_b)
    results = _run(in_maps)
    out = np.empty((G, 1, 1, H, W), np.float32)
    for c in range(8):
        o = results[c]["o"].reshape(G, 32, 256)
        out[:, 0, 0, 32 * c:32 * c + 32, :] = o
    out += np.asarray(t5_b).reshape(G, 1, 1, 1, 1)
    return out
